# revision 31
# baseline (speedup 1.0000x reference)
"""GroupedQueryAttention Trainium2 kernel (v3).

B=2, S=2048, D_MODEL=2048, 32 query heads / 8 KV heads, d_k=64.
Sharding: 8 cores = 2 (batch) x 4 (head groups of 8 query heads / 2 KV heads).
Per core: Wq/Wk/Wv column shard, Wo row shard; host sums the 4 partial
outputs per batch (the "all-reduce" of the row-parallel output projection).

v3 redesign (from v2's 414us trace): the ACT engine's exp stream is the
hard floor (256 instrs x ~1.15us = 278us busy, and it ran back-to-back
within pairs already). v2 lost ~76us of ACT stalls at pair/qt boundaries
(solid k/v/q/o-proj blocks + ctx tails emitted between pairs starved the
scores->exp chain) plus a 28.6us prologue and 33.5us tail. v3:
 - credit-paced emission: scores+exp are the metronome; everything else
   (ctx matmuls, projections, o_proj chunks, normalization) is split into
   ~0.5-2us units drained from deferred queues between kt steps, sized so
   PE never blocks the exp chain at a boundary.
 - ctx matmuls may lag their pair by many kts (deep e-tile ring in SBUF),
   so k/v-proj fills in qt0 and pair handoffs never stall ACT.
 - input DMA issue split across both HWDGE rings (x slabs on sync,
   weights on scalar) — v2 serialized 18 descriptors x 600ns on one ring,
   which alone delayed the first k-proj ~8us. x slabs land half-slab at a
   time so k-proj(0) starts ~3us in, under HAM-warming filler matmuls.
 - o_proj chunks gate on the previous qt's 4 normalizations and drain as
   ordinary fill units; qt3 drains hot so the tail is norm + 16 chunks.

Same numerics as v2: f16 inputs/stationaries, f32 PSUM, softmax denom
rides the ctx matmul's M dim (64 ones cols), 1/den via a bitwise-NOT-
seeded Newton step on DVE, f16 output partials summed on host in f64.

Dead ends (project memory): fp8/DoubleRow (3.2e-2 vs the 2e-2 gate),
f16 PSUM accumulation (TRN3-only — bass asserts fp32 out on TRN2),
ACT instruction merging (needs 8 PSUM banks of scores live; cap is 8
total), ctx col-group tetris (den+dims fixed tile_position packing needs
5-6 PSUM banks).
"""

import sys

sys.path.insert(0, "/opt/trn_rl_repo")

from collections import deque

import numpy as np

import concourse.bass as bass
import concourse.tile as tile
from concourse import bacc, mybir
from concourse.bass_utils import run_bass_kernel_spmd
from concourse.masks import make_identity

F32 = mybir.dt.float32
F16 = mybir.dt.float16

D = 2048          # d_model
S = 2048          # sequence length
HL = 8            # query heads per core
KVL = 2           # kv heads per core
DK = 64
QO = HL * DK      # 512 query outdims per core
KO = KVL * DK     # 128 kv outdims per core
NKT = 16          # d_model contraction tiles of 128
NTT = 16          # token tiles of 128
NQT = 4           # query tiles of 512
RC0 = 0.23583660820306038  # f16 NOT-seed reciprocal scale (y0 = -RC0*~den)
RC1 = 2.002                # first Newton constant (tuned for the f16 seed)

# emission credit model (ns estimates; only ratios matter)
MMC = 230         # one N=512 matmul in-stream
SCORES_COST = 300 # one paired scores step (2 concurrent MMs + LDW exposure)
ACT_NS = 1150     # one [128,2,512] exp instruction
CTX_COST = 250    # one col-tiled ctx slot (2 concurrent M=64 MMs)
DEN_COST = 250    # one den slot (4 concurrent M=32 ones MMs, 2 kts)

_CACHE = {}


def _build_nc():
    nc = bacc.Bacc("TRN2", target_bir_lowering=False)

    # host-prepacked layouts: contiguous per-partition runs on both DMA sides
    xT_h = nc.dram_tensor("xT", [128, 4, NKT, 512], F16, kind="ExternalInput")
    wq_h = nc.dram_tensor("wq", [128, 4, NKT, 128], F16, kind="ExternalInput")
    wk_h = nc.dram_tensor("wk", [128, NKT, KO], F16, kind="ExternalInput")
    wv_h = nc.dram_tensor("wv", [128, NKT, KO], F16, kind="ExternalInput")
    wo_h = nc.dram_tensor("wo", [QO, D], F16, kind="ExternalInput")
    bq_h = nc.dram_tensor("bq2", [128, 4], F32, kind="ExternalInput")
    bk_h = nc.dram_tensor("bk2", [128, 1], F32, kind="ExternalInput")
    bv_h = nc.dram_tensor("bv2", [128, 1], F32, kind="ExternalInput")
    out_h = nc.dram_tensor("out", [S, D], F16, kind="ExternalOutput")

    with tile.TileContext(nc) as tc:
        _emit(nc, tc, xT_h, wq_h, wk_h, wv_h, wo_h, bq_h, bk_h, bv_h, out_h)
    nc.compile()
    return nc


def _emit(nc, tc, xT_h, wq_h, wk_h, wv_h, wo_h, bq_h, bk_h, bv_h, out_h):
    from contextlib import ExitStack

    ctx = ExitStack()
    with ctx:
        consts = ctx.enter_context(tc.tile_pool(name="consts", bufs=1))
        persist = ctx.enter_context(tc.tile_pool(name="persist", bufs=1))
        # PSUM budget (8 banks): sp 2x2 + ctx 2 + pp 2
        spp = ctx.enter_context(tc.tile_pool(name="spp", bufs=2, space="PSUM"))
        ctxp = ctx.enter_context(tc.tile_pool(name="ctxp", bufs=1, space="PSUM"))
        ppp = ctx.enter_context(tc.tile_pool(name="ppp", bufs=2, space="PSUM"))
        EP_BUFS = 15
        ep = ctx.enter_context(tc.tile_pool(name="ep", bufs=EP_BUFS))
        rp = ctx.enter_context(tc.tile_pool(name="rp", bufs=3))
        nw = ctx.enter_context(tc.tile_pool(name="nw", bufs=2))
        op = ctx.enter_context(tc.tile_pool(name="op", bufs=2))

        ident = consts.tile([128, 128], F16)
        make_identity(nc, ident)

        # persistent SBUF
        wq_sb = persist.tile([128, 4, NKT, 128], F16)  # m-tile major
        wk_sb = persist.tile([128, NKT, KO], F16)
        wv_sb = persist.tile([128, NKT, KO], F16)
        wo_sb = persist.tile([128, 4, D], F16)
        # x slabs as half-tiles so k-proj can start early; slab0's first
        # half is further quartered so k-proj(0) starts after 0.5MB of DMA
        xtq = [persist.tile([128, 4, 512], F16, name=f"xtq{q}")
               for q in range(2)]
        xth = [
            (persist.tile([128, 8, 512], F16, name=f"xt{i}h{h}")
             if (i, h) != (0, 0) else None)
            for i in range(4) for h in range(2)
        ]
        qt_sb = persist.tile([128, 4, S], F16)     # [dim-in-pair, pair, token]
        ktd_sb = persist.tile([128, KVL, S], F16)  # kv dims dup'd both halves
        vt_sb = persist.tile([128, S], F16)        # [kv dims (2x64), token]
        # ctx lhsT: [token-in-128, key tile, kv, 64 V dims]
        vv_sb = persist.tile([128, NTT, KVL, 64], F16)
        ones_sb = persist.tile([128, 32], F16)     # den lhsT
        ctxT_sb = persist.tile([128, 4, S], F16)   # [dim-in-pair, pair, token]

        bq_sb = consts.tile([128, 4], F32)
        bk_sb = consts.tile([128, 1], F32)
        bv_sb = consts.tile([128, 1], F32)

        def xs(nt, kt):
            if nt == 0 and kt < 8:
                return xtq[kt // 4][:, kt % 4, :]
            return xth[nt * 2 + kt // 8][:, kt % 8, :]

        # --- input DMA: x slabs on the sync HWDGE ring, weights on the
        # scalar ring (issues run before the first activation). Ordered by
        # first-need so the prologue is not issue-serialized.
        nc.sync.dma_start(out=xtq[0], in_=xT_h[:, 0, 0:4])
        nc.sync.dma_start(out=xtq[1], in_=xT_h[:, 0, 4:8])
        nc.sync.dma_start(out=xth[1], in_=xT_h[:, 0, 8:16])
        nc.sync.dma_start(out=xth[2], in_=xT_h[:, 1, 0:8])
        nc.sync.dma_start(out=xth[3], in_=xT_h[:, 1, 8:16])
        nc.sync.dma_start(out=xth[4], in_=xT_h[:, 2, 0:8])
        nc.sync.dma_start(out=xth[5], in_=xT_h[:, 2, 8:16])
        nc.sync.dma_start(out=xth[6], in_=xT_h[:, 3, 0:8])
        nc.sync.dma_start(out=xth[7], in_=xT_h[:, 3, 8:16])
        nc.scalar.dma_start(out=wk_sb, in_=wk_h[:])
        nc.scalar.dma_start(out=wq_sb[:, 0], in_=wq_h[:, 0])
        nc.scalar.dma_start(out=bq_sb, in_=bq_h[:])
        nc.scalar.dma_start(out=bk_sb, in_=bk_h[:])
        nc.scalar.dma_start(out=bv_sb, in_=bv_h[:])
        nc.scalar.dma_start(out=wq_sb[:, 1], in_=wq_h[:, 1])
        nc.scalar.dma_start(out=wv_sb, in_=wv_h[:])
        nc.scalar.dma_start(out=wq_sb[:, 2], in_=wq_h[:, 2])
        nc.scalar.dma_start(out=wq_sb[:, 3], in_=wq_h[:, 3])
        nc.scalar.dma_start(
            out=wo_sb, in_=wo_h.rearrange("(c p) d -> p c d", p=128))

        nc.vector.memset(ones_sb, 1.0)  # den matmul stationary

        # ================= deferred emission machinery =================
        fills = deque()   # (cost, fn, ready) — strict FIFO
        ctxq = deque()    # ctx accumulation + norm units, may lag pairs
        credit = [0.0]
        emitted = {}      # group name -> units emitted
        total = {}        # group name -> units pushed
        vv_done = [False] * NTT
        norm_done = [0] * NQT
        acts_n = [0]   # e tiles allocated
        ctx_n = [0]    # ctx units emitted (e tiles released)

        def push(q, cost, fn, group=None, ready=None):
            if group is not None:
                total[group] = total.get(group, 0) + 1

            def fn2(fn=fn, group=group):
                fn()
                if group is not None:
                    emitted[group] = emitted.get(group, 0) + 1
            q.append((cost, fn2, ready or (lambda: True)))

        def pump():
            """Emit one deferred unit if possible. Returns cost or None."""
            # prefer ctx when the lag queue is deep (bounds the e ring)
            order = (ctxq, fills) if len(ctxq) >= 5 else (fills, ctxq)
            for q in order:
                if q and q[0][2]():
                    cost, fn, _ = q.popleft()
                    fn()
                    return cost
            return None

        def drain(budget):
            credit[0] += budget
            while credit[0] > 0:
                cost = pump()
                if cost is None:
                    break
                credit[0] -= cost

        def force_group(group):
            while emitted.get(group, 0) < total.get(group, 0):
                if pump() is None:
                    raise RuntimeError(f"stuck forcing {group}")

        def flush_all():
            while ctxq or fills:
                if pump() is None:
                    raise RuntimeError("stuck flushing")
            credit[0] = 0.0

        # ================= unit factories =================
        def push_kproj(nt):
            st = {}
            ns = slice(nt * 512, (nt + 1) * 512)

            def mk(c0):
                def fn():
                    if c0 == 0:
                        st['ps'] = ppp.tile([128, 512], F32, tag="pp", name="pp")
                    for kt in range(c0, c0 + 4):
                        nc.tensor.matmul(
                            st['ps'], lhsT=wk_sb[:, kt, :], rhs=xs(nt, kt),
                            start=(kt == 0), stop=(kt == NKT - 1))
                return fn
            for c0 in (0, 4, 8, 12):
                push(fills, 4 * MMC, mk(c0), group=f"kp{nt}")

            def fin():
                kb = rp.tile([128, 512], F16, tag="kb")
                nc.vector.tensor_scalar_add(kb, st['ps'], bk_sb[:, 0:1])
                for kv in range(KVL):
                    src = kb[kv * 64:(kv + 1) * 64, 0:512]
                    nc.vector.tensor_copy(ktd_sb[0:64, kv, ns], src)
                    nc.vector.tensor_copy(ktd_sb[64:128, kv, ns], src)
            push(fills, 60, fin, group=f"kp{nt}")

        def push_qproj(qt, mt):
            st = {}
            ns = slice(qt * 512, (qt + 1) * 512)
            g = f"qp{qt}_{mt}"

            def mk(c0):
                def fn():
                    if c0 == 0:
                        st['ps'] = ppp.tile([128, 512], F32, tag="pp", name="pp")
                    for kt in range(c0, c0 + 4):
                        nc.tensor.matmul(
                            st['ps'], lhsT=wq_sb[:, mt, kt, :],
                            rhs=xs(qt, kt),
                            start=(kt == 0), stop=(kt == NKT - 1))
                return fn
            for c0 in (0, 4, 8, 12):
                push(fills, 4 * MMC, mk(c0), group=g)

            def fin():
                nc.vector.tensor_scalar_add(
                    qt_sb[:, mt, ns], st['ps'], bq_sb[:, mt:mt + 1])
            push(fills, 60, fin, group=g)

        def push_vproj(nt):
            st = {}
            ns = slice(nt * 512, (nt + 1) * 512)

            def mk(c0):
                def fn():
                    if c0 == 0:
                        st['ps'] = ppp.tile([128, 512], F32, tag="pp", name="pp")
                    for kt in range(c0, c0 + 4):
                        nc.tensor.matmul(
                            st['ps'], lhsT=wv_sb[:, kt, :], rhs=xs(nt, kt),
                            start=(kt == 0), stop=(kt == NKT - 1))
                return fn
            for c0 in (0, 4, 8, 12):
                push(fills, 4 * MMC, mk(c0))

            def fin():
                nc.vector.tensor_scalar_add(vt_sb[:, ns], st['ps'], bv_sb[:, 0:1])
            push(fills, 60, fin)

            # V^T -> natural V layout via PE transpose into the ctx lhsT
            def mkt(tt):
                def fn():
                    pst = ppp.tile([128, 128], F16, tag="pp", name="pst")
                    nc.tensor.transpose(
                        pst[:, 0:128], vt_sb[:, tt * 128:(tt + 1) * 128],
                        ident[:])
                    for kv in range(KVL):
                        nc.vector.tensor_copy(
                            vv_sb[:, tt, kv, 0:64],
                            pst[:, kv * 64:(kv + 1) * 64])
                    vv_done[tt] = True
                return fn
            for tt in range(4 * nt, 4 * nt + 4):
                push(fills, 260, mkt(tt))

        def push_oproj(qt):
            st = {}
            g = f"op{qt}"
            rdy = (lambda qt=qt: norm_done[qt] == 4)
            for tl in range(4):
                for dn in range(4):
                    def fn(tl=tl, dn=dn):
                        ts_ = slice(qt * 512 + tl * 128,
                                    qt * 512 + (tl + 1) * 128)
                        ds_ = slice(dn * 512, (dn + 1) * 512)
                        ps = ppp.tile([128, 512], F32, tag="pp", name="pp")
                        for c in range(4):
                            nc.tensor.matmul(
                                ps, lhsT=ctxT_sb[:, c, ts_],
                                rhs=wo_sb[:, c, ds_],
                                start=(c == 0), stop=(c == 3))
                        if dn % 2 == 0:
                            st['ob'] = op.tile([128, 1024], F16, tag="ob", name="ob")
                        nc.vector.tensor_copy(
                            st['ob'][:, (dn % 2) * 512:(dn % 2) * 512 + 512],
                            ps)
                        if dn % 2 == 1:
                            nc.sync.dma_start(
                                out=out_h[ts_, (dn - 1) * 512:(dn + 1) * 512],
                                in_=st['ob'])
                    push(fills, 4 * MMC + 90, fn, group=g, ready=rdy)

        # ctx + normalization (lag queue)
        ctx_st = {}

        def push_ctx_block(qt, pair, kt2, e_lo, e_hi):
            # both heads col-tiled into one [128,512] bank: head i occupies
            # partitions 64i:64i+64 via tile_position (0, 64i) and the MMs
            # run concurrently on disjoint col groups. start=True is
            # per-REGION (HW-verified: the has_written clear is scoped to
            # the instruction's output region, not the whole bank). Two kts
            # per unit keep the (128,64) tile geometry resident.
            kv = pair // 2

            def fn():
                if kt2 == 0:
                    ctx_st[(qt, pair)] = (
                        ctxp.tile([128, 512], F32, tag="ctx", name="ctx"),
                        ctxp.tile([128, 512], F32, tag="den", name="den"),
                    )
                cps = ctx_st[(qt, pair)][0]
                for kt, e in ((kt2, e_lo), (kt2 + 1, e_hi)):
                    for i in range(2):
                        nc.tensor.matmul(
                            cps[i * 64:(i + 1) * 64, :],
                            lhsT=vv_sb[:, kt, kv, :],
                            rhs=e[:, i, :],
                            start=(kt == 0), stop=(kt == NTT - 1),
                            tile_position=(0, i * 64),
                            skip_group_check=True,
                        )
            push(ctxq, 2 * CTX_COST, fn,
                 ready=(lambda kt2=kt2: vv_done[kt2] and vv_done[kt2 + 1]))

        def push_den_block(qt, pair, kt8, es):
            # softmax denominators for kts kt8..kt8+7 (es = 4 e tiles of 2
            # kts each): 4 quads of concurrent M=32 ones-matmuls, emitted
            # contiguously so the (128,32) tile-geometry switch is paid once.
            # Region layout: [0:32]=h0 even-kt, [32:64]=h1 even, [64:96]=h0
            # odd, [96:128]=h1 odd; norm sums even+odd halves.
            def fn():
                dps = ctx_st[(qt, pair)][1]
                for q, e2 in enumerate(es):
                    kt2 = kt8 + 2 * q
                    for j, (lo, i) in enumerate(
                            [(0, 0), (0, 1), (1, 0), (1, 1)]):
                        nc.tensor.matmul(
                            dps[j * 32:(j + 1) * 32, :],
                            lhsT=ones_sb[:, 0:32],
                            rhs=e2[lo][:, i, :],
                            start=(kt2 == 0),
                            stop=(kt2 == NTT - 2 and j == 3),
                            tile_position=(0, j * 32),
                            skip_group_check=True,
                        )
                ctx_n[0] += 8
            push(ctxq, 4 * DEN_COST, fn)

        def push_norm(qt, pair):
            qs = slice(qt * 512, (qt + 1) * 512)

            def fn():
                cps, dps = ctx_st.pop((qt, pair))
                # evict PSUM fast (frees ctx banks), then normalize from SBUF.
                # 1/den via NOT-seeded Newton (6 short DVE ops per head
                # instead of one 3.3us RECIPROCAL head-of-line block).
                cus = []
                for i in range(2):
                    cu = rp.tile([64, 512], F16, tag="cu")
                    nc.vector.tensor_copy(cu, cps[i * 64:(i + 1) * 64, :])
                    cus.append(cu)
                # den halves: [0:64]=even kts [h0|h1], [64:128]=odd kts
                do = rp.tile([64, 512], F16, tag="do")
                nc.vector.tensor_copy(do, dps[64:128, :])
                ds = rp.tile([64, 512], F16, tag="ds")
                nc.vector.tensor_tensor(
                    ds, dps[0:64, :], do, mybir.AluOpType.add)
                MULT, ADD = mybir.AluOpType.mult, mybir.AluOpType.add
                for i in range(2):
                    # den replicated to 64 rows at base partition 0 (walrus
                    # requires equal SB base partitions for two-tensor ops)
                    den = nw.tile([64, 512], F16, tag="den")
                    nc.vector.tensor_copy(den[0:32, :], ds[i * 32:(i + 1) * 32, :])
                    nc.vector.tensor_copy(den[32:64, :], ds[i * 32:(i + 1) * 32, :])
                    n = nw.tile([64, 512], F16, tag="nt")
                    # seed: bits(n) = ~bits(den); y0 = C0*n approximates 1/den
                    nc.vector.tensor_scalar(
                        n.bitcast(mybir.dt.int16),
                        den.bitcast(mybir.dt.int16),
                        -1, None, mybir.AluOpType.bitwise_xor)
                    s1 = nw.tile([64, 512], F16, tag="s1")
                    nc.vector.scalar_tensor_tensor(s1, den, RC0, n, MULT, MULT)
                    s2 = nw.tile([64, 512], F16, tag="s2")
                    nc.vector.scalar_tensor_tensor(s2, s1, RC1, n, ADD, MULT)
                    # y1 = (-RC0)*s2 ~ 1/den; ctxT = ctx * y1
                    nc.vector.scalar_tensor_tensor(
                        ctxT_sb[i * 64:(i + 1) * 64, pair, qs],
                        cus[i], -RC0, s2, MULT, MULT)
                norm_done[qt] += 1
            push(ctxq, 120, fn)

        # ================= scores / exp step =================
        def scores_step(qt, pair, kt):
            # e-ring liveness: the next ep.tile() may reuse a slot whose ctx
            # read hasn't been EMITTED yet (ring WAR is only tracked against
            # already-emitted readers) — drain ctx until the ring has slack.
            while acts_n[0] - ctx_n[0] >= EP_BUFS - 2:
                if pump() is None:
                    raise RuntimeError("stuck draining e ring")
            qs = slice(qt * 512, (qt + 1) * 512)
            ks = slice(kt * 128, (kt + 1) * 128)
            kv = pair // 2
            sp = spp.tile([128, 2, 512], F32, tag="sp")
            for i in range(2):
                nc.tensor.matmul(
                    sp[:, i, :],
                    lhsT=ktd_sb[i * 64:(i + 1) * 64, kv, ks],
                    rhs=qt_sb[i * 64:(i + 1) * 64, pair, qs],
                    start=True, stop=True,
                    tile_position=(i * 64, 0),
                )
            e = ep.tile([128, 2, 512], F16, tag="e")
            nc.scalar.activation(
                e[:, :, :], sp[:, :, :],
                mybir.ActivationFunctionType.Exp, scale=0.125)
            acts_n[0] += 1
            return e

        # ================= prologue =================
        # HAM warm-up: >=3.4us of continuous N=512 matmuls from ~t0 so the
        # PE clock is at 8/8 when k-proj's DMA lands (v3.2 used 16 N=128
        # MMs = 1.9us busy — every prologue MM then ran at 1.2 GHz).
        wconst = consts.tile([128, 512], F16)
        nc.vector.memset(wconst, 0.25)
        wps = ppp.tile([128, 512], F32, tag="pp")
        for _ in range(14):
            nc.tensor.matmul(wps, lhsT=ident[:], rhs=wconst[:],
                             start=True, stop=True)
        # k_proj(0) + q_proj(0,0) inline (gate the first scores)
        push_kproj(0)
        force_group("kp0")
        push_qproj(0, 0)
        force_group("qp0_0")
        push_qproj(0, 1)  # drains during pair 0, forced at pair 1 start

        # ================= main attention stream =================
        for qt in range(NQT):
            for pair in range(4):
                g = qt * 4 + pair
                # this pair's q-proj must be fully emitted before its scores
                force_group(f"qp{qt}_{pair}")
                # releases: q-projs ahead of (gated) o_proj units in FIFO so
                # pair-start forces never drain an o_proj backlog first
                if pair == 0 and g + 3 < 16:
                    push_qproj((g + 2) // 4, (g + 2) % 4)
                    push_qproj((g + 3) // 4, (g + 3) % 4)
                elif pair == 2 and g + 2 < 16:
                    push_qproj((g + 2) // 4, (g + 2) % 4)
                    if g + 3 < 16:
                        push_qproj((g + 3) // 4, (g + 3) % 4)
                if pair == 0 and qt >= 1:
                    push_oproj(qt - 1)
                den_es = []
                for kt2 in range(0, NTT, 2):
                    if qt == 0 and pair == 0:
                        if kt2 == 0:
                            push_kproj(1)
                            push_kproj(2)
                            push_kproj(3)
                        if kt2 in (2, 6, 10, 14):
                            push_vproj((kt2 - 2) // 4)
                        if kt2 in (4, 8, 12):
                            force_group(f"kp{kt2 // 4}")
                    # two kts back-to-back: same (64,128) tile geometry, so
                    # the second kt's LDWs overlap the first's in-flight MMs
                    e_lo = scores_step(qt, pair, kt2)
                    e_hi = scores_step(qt, pair, kt2 + 1)
                    push_ctx_block(qt, pair, kt2, e_lo, e_hi)
                    den_es.append((e_lo, e_hi))
                    if kt2 % 8 == 6:
                        push_den_block(qt, pair, kt2 - 6, den_es)
                        den_es = []
                    drain(2 * (ACT_NS - SCORES_COST))
                push_norm(qt, pair)

        # ================= tail =================
        flush_all()
        # warmth bridge: keep the PE HAM clock up through the final
        # normalization gap so o_proj(3) doesn't run cold
        wps2 = ppp.tile([128, 512], F32, tag="pp")
        for _ in range(8):
            nc.tensor.matmul(wps2[:, 0:128], lhsT=ident[:], rhs=ident[:],
                             start=True, stop=True)
        push_oproj(3)
        force_group("op3")


def _get_nc():
    if "nc" not in _CACHE:
        _CACHE["nc"] = _build_nc()
    return _CACHE["nc"]


def _pack_x(x_b):
    """[S, D] -> [128, 4 slab, 16 kt, 512 t]: xT[k*128+p, nt*512+t]."""
    xT = x_b.T.astype(np.float16)                       # [D, S]
    return np.ascontiguousarray(
        xT.reshape(NKT, 128, 4, 512).transpose(1, 2, 0, 3))


def _pack_wq(wq_s):
    """[D, 512] -> [128, 4 mt, 16 kt, 128]: wq[k*128+p, mt*128+m]."""
    w = wq_s.astype(np.float16)
    return np.ascontiguousarray(
        w.reshape(NKT, 128, 4, 128).transpose(1, 2, 0, 3))


def _pack_wkv(w_s):
    """[D, 128] -> [128, 16 kt, 128]: w[k*128+p, m]."""
    w = w_s.astype(np.float16)
    return np.ascontiguousarray(w.reshape(NKT, 128, KO).transpose(1, 0, 2))


def kernel(x, Wq, bq, Wk, bk, Wv, bv, Wo, bo, _trace=False):
    x = np.asarray(x, np.float32)
    Wq = np.asarray(Wq, np.float32)
    bq = np.asarray(bq, np.float32)
    Wk = np.asarray(Wk, np.float32)
    bk = np.asarray(bk, np.float32)
    Wv = np.asarray(Wv, np.float32)
    bv = np.asarray(bv, np.float32)
    Wo = np.asarray(Wo, np.float32)
    bo = np.asarray(bo, np.float32)

    nc = _get_nc()
    in_maps = []
    for r in range(8):
        b, g = divmod(r, 4)
        qsl = slice(g * 512, (g + 1) * 512)
        ksl = slice(g * 128, (g + 1) * 128)
        in_maps.append({
            "xT": _pack_x(x[b]),
            "wq": _pack_wq(Wq[:, qsl]),
            "wk": _pack_wkv(Wk[:, ksl]),
            "wv": _pack_wkv(Wv[:, ksl]),
            "wo": np.ascontiguousarray(Wo[qsl, :].astype(np.float16)),
            "bq2": np.ascontiguousarray(bq[qsl].reshape(4, 128).T),
            "bk2": np.ascontiguousarray(bk[ksl].reshape(128, 1)),
            "bv2": np.ascontiguousarray(bv[ksl].reshape(128, 1)),
        })

    res = run_bass_kernel_spmd(nc, in_maps, list(range(8)), trace=_trace)
    out = np.zeros((2, S, D), np.float64)
    for r in range(8):
        out[r // 4] += res.results[r]["out"].astype(np.float64)
    out += bo.astype(np.float64)
    result = out.astype(np.float32)
    if _trace:
        return result, res
    return result


# revision 33
# speedup vs baseline: 1.0980x; 1.0980x over previous
"""GroupedQueryAttention Trainium2 kernel (v3).

B=2, S=2048, D_MODEL=2048, 32 query heads / 8 KV heads, d_k=64.
Sharding: 8 cores = 2 (batch) x 4 (head groups of 8 query heads / 2 KV heads).
Per core: Wq/Wk/Wv column shard, Wo row shard; host sums the 4 partial
outputs per batch (the "all-reduce" of the row-parallel output projection).

v3 redesign (from v2's 414us trace): the ACT engine's exp stream is the
hard floor (256 instrs x ~1.15us = 278us busy, and it ran back-to-back
within pairs already). v2 lost ~76us of ACT stalls at pair/qt boundaries
(solid k/v/q/o-proj blocks + ctx tails emitted between pairs starved the
scores->exp chain) plus a 28.6us prologue and 33.5us tail. v3:
 - credit-paced emission: scores+exp are the metronome; everything else
   (ctx matmuls, projections, o_proj chunks, normalization) is split into
   ~0.5-2us units drained from deferred queues between kt steps, sized so
   PE never blocks the exp chain at a boundary.
 - ctx matmuls may lag their pair by many kts (deep e-tile ring in SBUF),
   so k/v-proj fills in qt0 and pair handoffs never stall ACT.
 - input DMA issue split across both HWDGE rings (x slabs on sync,
   weights on scalar) — v2 serialized 18 descriptors x 600ns on one ring,
   which alone delayed the first k-proj ~8us. x slabs land half-slab at a
   time so k-proj(0) starts ~3us in, under HAM-warming filler matmuls.
 - o_proj chunks gate on the previous qt's 4 normalizations and drain as
   ordinary fill units; qt3 drains hot so the tail is norm + 16 chunks.

Same numerics as v2: f16 inputs/stationaries, f32 PSUM, softmax denom
rides the ctx matmul's M dim (64 ones cols), 1/den via a bitwise-NOT-
seeded Newton step on DVE, f16 output partials summed on host in f64.

Dead ends (project memory): fp8/DoubleRow (3.2e-2 vs the 2e-2 gate),
f16 PSUM accumulation (TRN3-only — bass asserts fp32 out on TRN2),
ACT instruction merging (needs 8 PSUM banks of scores live; cap is 8
total), ctx col-group tetris (den+dims fixed tile_position packing needs
5-6 PSUM banks).
"""

import sys

sys.path.insert(0, "/opt/trn_rl_repo")

from collections import deque

import numpy as np

import concourse.bass as bass
import concourse.tile as tile
from concourse import bacc, mybir
from concourse.bass_utils import run_bass_kernel_spmd
from concourse.masks import make_identity

F32 = mybir.dt.float32
F16 = mybir.dt.float16

D = 2048          # d_model
S = 2048          # sequence length
HL = 8            # query heads per core
KVL = 2           # kv heads per core
DK = 64
QO = HL * DK      # 512 query outdims per core
KO = KVL * DK     # 128 kv outdims per core
NKT = 16          # d_model contraction tiles of 128
NTT = 16          # token tiles of 128
NQT = 4           # query tiles of 512
RC0 = 0.23583660820306038  # f16 NOT-seed reciprocal scale (y0 = -RC0*~den)
RC1 = 2.002                # first Newton constant (tuned for the f16 seed)

# emission credit model (ns estimates; only ratios matter)
MMC = 230         # one N=512 matmul in-stream
SCORES_COST = 300 # one paired scores step (2 concurrent MMs + LDW exposure)
ACT_NS = 1150     # one [128,2,512] exp instruction
CTX_COST = 250    # one col-tiled ctx slot (2 concurrent M=64 MMs)
DEN_COST = 250    # one den slot (4 concurrent M=32 ones MMs, 2 kts)

_CACHE = {}


def _build_nc():
    nc = bacc.Bacc("TRN2", target_bir_lowering=False)

    # host-prepacked layouts: contiguous per-partition runs on both DMA sides
    xT_h = nc.dram_tensor("xT", [128, 4, NKT, 512], F16, kind="ExternalInput")
    wq_h = nc.dram_tensor("wq", [128, 4, NKT, 128], F16, kind="ExternalInput")
    wk_h = nc.dram_tensor("wk", [128, NKT, KO], F16, kind="ExternalInput")
    wv_h = nc.dram_tensor("wv", [128, NKT, KO], F16, kind="ExternalInput")
    wo_h = nc.dram_tensor("wo", [QO, D], F16, kind="ExternalInput")
    bq_h = nc.dram_tensor("bq2", [128, 4], F32, kind="ExternalInput")
    bk_h = nc.dram_tensor("bk2", [128, 1], F32, kind="ExternalInput")
    bv_h = nc.dram_tensor("bv2", [128, 1], F32, kind="ExternalInput")
    out_h = nc.dram_tensor("out", [S, D], F16, kind="ExternalOutput")

    with tile.TileContext(nc) as tc:
        _emit(nc, tc, xT_h, wq_h, wk_h, wv_h, wo_h, bq_h, bk_h, bv_h, out_h)
    nc.compile()
    return nc


def _emit(nc, tc, xT_h, wq_h, wk_h, wv_h, wo_h, bq_h, bk_h, bv_h, out_h):
    from contextlib import ExitStack

    ctx = ExitStack()
    with ctx:
        consts = ctx.enter_context(tc.tile_pool(name="consts", bufs=1))
        persist = ctx.enter_context(tc.tile_pool(name="persist", bufs=1))
        # PSUM budget (8 banks): sp 2x2 + ctx 2 + pp 2
        spp = ctx.enter_context(tc.tile_pool(name="spp", bufs=2, space="PSUM"))
        ctxp = ctx.enter_context(tc.tile_pool(name="ctxp", bufs=1, space="PSUM"))
        ppp = ctx.enter_context(tc.tile_pool(name="ppp", bufs=2, space="PSUM"))
        EP_BUFS = 15
        ep = ctx.enter_context(tc.tile_pool(name="ep", bufs=EP_BUFS))
        rp = ctx.enter_context(tc.tile_pool(name="rp", bufs=3))
        nw = ctx.enter_context(tc.tile_pool(name="nw", bufs=2))
        op = ctx.enter_context(tc.tile_pool(name="op", bufs=3))

        ident = consts.tile([128, 128], F16)
        make_identity(nc, ident)

        # persistent SBUF
        wq_sb = persist.tile([128, 4, NKT, 128], F16)  # m-tile major
        wk_sb = persist.tile([128, NKT, KO], F16)
        wv_sb = persist.tile([128, NKT, KO], F16)
        wo_sb = persist.tile([128, 4, D], F16)
        # x slabs as half-tiles so k-proj(0) can start after 1MB of DMA
        xth = [
            persist.tile([128, 8, 512], F16, name=f"xt{i}h{h}")
            for i in range(4) for h in range(2)
        ]
        qt_sb = persist.tile([128, 4, S], F16)     # [dim-in-pair, pair, token]
        ktd_sb = persist.tile([128, KVL, S], F16)  # kv dims dup'd both halves
        vt_sb = persist.tile([128, S], F16)        # [kv dims (2x64), token]
        # ctx lhsT: [token-in-128, key tile, kv, 64 V dims | 64 ones]
        vv_sb = persist.tile([128, NTT, KVL, 128], F16)
        ctxT_sb = persist.tile([128, 4, S], F16)   # [dim-in-pair, pair, token]

        bq_sb = consts.tile([128, 4], F32)
        bk_sb = consts.tile([128, 1], F32)
        bv_sb = consts.tile([128, 1], F32)

        def xs(nt, kt):
            return xth[nt * 2 + kt // 8][:, kt % 8, :]

        # --- input DMA: x slabs on the sync HWDGE ring, weights on the
        # scalar ring (issues run before the first activation). Ordered by
        # first-need so the prologue is not issue-serialized.
        nc.sync.dma_start(out=xth[0], in_=xT_h[:, 0, 0:8])
        nc.sync.dma_start(out=xth[1], in_=xT_h[:, 0, 8:16])
        nc.sync.dma_start(out=xth[2], in_=xT_h[:, 1, 0:8])
        nc.sync.dma_start(out=xth[3], in_=xT_h[:, 1, 8:16])
        nc.sync.dma_start(out=xth[4], in_=xT_h[:, 2, 0:8])
        nc.sync.dma_start(out=xth[5], in_=xT_h[:, 2, 8:16])
        nc.sync.dma_start(out=xth[6], in_=xT_h[:, 3, 0:8])
        nc.sync.dma_start(out=xth[7], in_=xT_h[:, 3, 8:16])
        nc.scalar.dma_start(out=wk_sb, in_=wk_h[:])
        nc.scalar.dma_start(out=bq_sb, in_=bq_h[:])
        nc.scalar.dma_start(out=bk_sb, in_=bk_h[:])
        nc.scalar.dma_start(out=bv_sb, in_=bv_h[:])
        nc.scalar.dma_start(out=wq_sb[:, 0], in_=wq_h[:, 0])
        nc.scalar.dma_start(out=wq_sb[:, 1], in_=wq_h[:, 1])
        nc.scalar.dma_start(out=wv_sb, in_=wv_h[:])
        nc.scalar.dma_start(out=wq_sb[:, 2], in_=wq_h[:, 2])
        nc.scalar.dma_start(out=wq_sb[:, 3], in_=wq_h[:, 3])
        nc.scalar.dma_start(
            out=wo_sb, in_=wo_h.rearrange("(c p) d -> p c d", p=128))

        nc.vector.memset(vv_sb, 1.0)  # ones cols; dim cols overwritten below

        # ================= deferred emission machinery =================
        fills = deque()   # (cost, fn, ready) — strict FIFO
        ctxq = deque()    # ctx accumulation + norm units, may lag pairs
        credit = [0.0]
        emitted = {}      # group name -> units emitted
        total = {}        # group name -> units pushed
        vv_done = [False] * NTT
        norm_done = [0] * NQT
        acts_n = [0]   # e tiles allocated
        ctx_n = [0]    # ctx units emitted (e tiles released)

        def push(q, cost, fn, group=None, ready=None):
            if group is not None:
                total[group] = total.get(group, 0) + 1

            def fn2(fn=fn, group=group):
                fn()
                if group is not None:
                    emitted[group] = emitted.get(group, 0) + 1
            q.append((cost, fn2, ready or (lambda: True)))

        def pump():
            """Emit one deferred unit if possible. Returns cost or None."""
            # prefer ctx when the lag queue is deep (bounds the e ring)
            order = (ctxq, fills) if len(ctxq) >= 5 else (fills, ctxq)
            for q in order:
                if q and q[0][2]():
                    cost, fn, _ = q.popleft()
                    fn()
                    return cost
            return None

        def drain(budget):
            credit[0] += budget
            while credit[0] > 0:
                cost = pump()
                if cost is None:
                    break
                credit[0] -= cost

        def force_group(group):
            while emitted.get(group, 0) < total.get(group, 0):
                if pump() is None:
                    raise RuntimeError(f"stuck forcing {group}")

        def flush_all():
            while ctxq or fills:
                if pump() is None:
                    raise RuntimeError("stuck flushing")
            credit[0] = 0.0

        # ================= unit factories =================
        def push_kproj(nt):
            st = {}
            ns = slice(nt * 512, (nt + 1) * 512)

            def mk(c0):
                def fn():
                    if c0 == 0:
                        st['ps'] = ppp.tile([128, 512], F32, tag="pp", name="pp")
                    for kt in range(c0, c0 + 4):
                        nc.tensor.matmul(
                            st['ps'], lhsT=wk_sb[:, kt, :], rhs=xs(nt, kt),
                            start=(kt == 0), stop=(kt == NKT - 1))
                return fn
            for c0 in (0, 4, 8, 12):
                push(fills, 4 * MMC, mk(c0), group=f"kp{nt}")

            def fin():
                kb = rp.tile([128, 512], F16, tag="kb")
                nc.vector.tensor_scalar_add(kb, st['ps'], bk_sb[:, 0:1])
                for kv in range(KVL):
                    src = kb[kv * 64:(kv + 1) * 64, 0:512]
                    nc.vector.tensor_copy(ktd_sb[0:64, kv, ns], src)
                    nc.vector.tensor_copy(ktd_sb[64:128, kv, ns], src)
            push(fills, 60, fin, group=f"kp{nt}")

        def push_qproj(qt, mt):
            st = {}
            ns = slice(qt * 512, (qt + 1) * 512)
            g = f"qp{qt}_{mt}"

            def mk(c0):
                def fn():
                    if c0 == 0:
                        st['ps'] = ppp.tile([128, 512], F32, tag="pp", name="pp")
                    for kt in range(c0, c0 + 4):
                        nc.tensor.matmul(
                            st['ps'], lhsT=wq_sb[:, mt, kt, :],
                            rhs=xs(qt, kt),
                            start=(kt == 0), stop=(kt == NKT - 1))
                return fn
            for c0 in (0, 4, 8, 12):
                push(fills, 4 * MMC, mk(c0), group=g)

            def fin():
                nc.vector.tensor_scalar_add(
                    qt_sb[:, mt, ns], st['ps'], bq_sb[:, mt:mt + 1])
            push(fills, 60, fin, group=g)

        def push_vproj(nt):
            st = {}
            ns = slice(nt * 512, (nt + 1) * 512)

            def mk(c0):
                def fn():
                    if c0 == 0:
                        st['ps'] = ppp.tile([128, 512], F32, tag="pp", name="pp")
                    for kt in range(c0, c0 + 4):
                        nc.tensor.matmul(
                            st['ps'], lhsT=wv_sb[:, kt, :], rhs=xs(nt, kt),
                            start=(kt == 0), stop=(kt == NKT - 1))
                return fn
            for c0 in (0, 4, 8, 12):
                push(fills, 4 * MMC, mk(c0))

            def fin():
                nc.vector.tensor_scalar_add(vt_sb[:, ns], st['ps'], bv_sb[:, 0:1])
            push(fills, 60, fin)

            # V^T -> natural V layout via PE transpose into the ctx lhsT
            def mkt(tt):
                def fn():
                    pst = ppp.tile([128, 128], F16, tag="pp", name="pst")
                    nc.tensor.transpose(
                        pst[:, 0:128], vt_sb[:, tt * 128:(tt + 1) * 128],
                        ident[:])
                    for kv in range(KVL):
                        nc.vector.tensor_copy(
                            vv_sb[:, tt, kv, 0:64],
                            pst[:, kv * 64:(kv + 1) * 64])
                    vv_done[tt] = True
                return fn
            for tt in range(4 * nt, 4 * nt + 4):
                push(fills, 260, mkt(tt))

        def push_oproj(qt):
            st = {}
            g = f"op{qt}"
            rdy = (lambda qt=qt: norm_done[qt] == 4)
            for tl in range(4):
                for dn in range(4):
                    def fn(tl=tl, dn=dn):
                        ts_ = slice(qt * 512 + tl * 128,
                                    qt * 512 + (tl + 1) * 128)
                        ds_ = slice(dn * 512, (dn + 1) * 512)
                        ps = ppp.tile([128, 512], F32, tag="pp", name="pp")
                        for c in range(4):
                            nc.tensor.matmul(
                                ps, lhsT=ctxT_sb[:, c, ts_],
                                rhs=wo_sb[:, c, ds_],
                                start=(c == 0), stop=(c == 3))
                        if dn % 2 == 0:
                            st['ob'] = op.tile([128, 1024], F16, tag="ob", name="ob")
                        nc.vector.tensor_copy(
                            st['ob'][:, (dn % 2) * 512:(dn % 2) * 512 + 512],
                            ps)
                        if dn % 2 == 1:
                            nc.sync.dma_start(
                                out=out_h[ts_, (dn - 1) * 512:(dn + 1) * 512],
                                in_=st['ob'])
                    push(fills, 4 * MMC + 90, fn, group=g, ready=rdy)

        # ctx + normalization (lag queue)
        ctx_st = {}

        def push_ctx(qt, pair, kt, e):
            kv = pair // 2

            def fn():
                if kt == 0:
                    ctx_st[(qt, pair)] = [
                        ctxp.tile([128, 512], F32, tag=f"ctx{i}", name=f"ctx{i}")
                        for i in range(2)
                    ]
                ps = ctx_st[(qt, pair)]
                for i in range(2):
                    nc.tensor.matmul(
                        ps[i][:, :],
                        lhsT=vv_sb[:, kt, kv, :],
                        rhs=e[:, i, :],
                        start=(kt == 0), stop=(kt == NTT - 1),
                    )
                ctx_n[0] += 1
            push(ctxq, CTX_COST, fn, ready=(lambda kt=kt: vv_done[kt]))

        def push_norm(qt, pair):
            qs = slice(qt * 512, (qt + 1) * 512)

            def fn():
                ps = ctx_st.pop((qt, pair))
                # evict PSUM fast (frees ctx banks), then normalize from SBUF.
                cus = []
                for i in range(2):
                    cu = rp.tile([128, 512], F16, tag="cu")
                    nc.vector.tensor_copy(cu, ps[i])
                    cus.append(cu)
                MULT, ADD = mybir.AluOpType.mult, mybir.AluOpType.add
                for i in range(2):
                    den = nw.tile([64, 512], F16, tag="den")
                    nc.vector.tensor_copy(den, cus[i][64:128, :])
                    n = nw.tile([64, 512], F16, tag="nt")
                    nc.vector.tensor_scalar(
                        n.bitcast(mybir.dt.int16),
                        den.bitcast(mybir.dt.int16),
                        -1, None, mybir.AluOpType.bitwise_xor)
                    s1 = nw.tile([64, 512], F16, tag="s1")
                    nc.vector.scalar_tensor_tensor(s1, den, RC0, n, MULT, MULT)
                    s2 = nw.tile([64, 512], F16, tag="s2")
                    nc.vector.scalar_tensor_tensor(s2, s1, RC1, n, ADD, MULT)
                    nc.vector.scalar_tensor_tensor(
                        ctxT_sb[i * 64:(i + 1) * 64, pair, qs],
                        cus[i][0:64, :], -RC0, s2, MULT, MULT)
                norm_done[qt] += 1
            push(ctxq, 120, fn)

        # ================= scores / exp step =================
        def scores_step(qt, pair, kt):
            # e-ring liveness: the next ep.tile() may reuse a slot whose ctx
            # read hasn't been EMITTED yet (ring WAR is only tracked against
            # already-emitted readers) — drain ctx until the ring has slack.
            while acts_n[0] - ctx_n[0] >= EP_BUFS - 2:
                if pump() is None:
                    raise RuntimeError("stuck draining e ring")
            qs = slice(qt * 512, (qt + 1) * 512)
            ks = slice(kt * 128, (kt + 1) * 128)
            kv = pair // 2
            sp = spp.tile([128, 2, 512], F32, tag="sp")
            for i in range(2):
                nc.tensor.matmul(
                    sp[:, i, :],
                    lhsT=ktd_sb[i * 64:(i + 1) * 64, kv, ks],
                    rhs=qt_sb[i * 64:(i + 1) * 64, pair, qs],
                    start=True, stop=True,
                    tile_position=(i * 64, 0),
                )
            e = ep.tile([128, 2, 512], F16, tag="e")
            nc.scalar.activation(
                e[:, :, :], sp[:, :, :],
                mybir.ActivationFunctionType.Exp, scale=0.125)
            acts_n[0] += 1
            return e

        # ================= prologue =================
        # HAM warm-up: PE busy from ~t0 while DMA lands; K=8/8 by k-proj.
        wps = ppp.tile([128, 512], F32, tag="pp")
        for _ in range(12):
            nc.tensor.matmul(wps[:, 0:128], lhsT=ident[:], rhs=ident[:],
                             start=True, stop=True)
        # k_proj(0) + q_proj(0,0) inline (gate the first scores)
        push_kproj(0)
        force_group("kp0")
        push_qproj(0, 0)
        force_group("qp0_0")
        push_qproj(0, 1)  # drains during pair 0, forced at pair 1 start

        # ================= main attention stream =================
        for qt in range(NQT):
            for pair in range(4):
                g = qt * 4 + pair
                # this pair's q-proj must be fully emitted before its scores
                force_group(f"qp{qt}_{pair}")
                g2 = g + 2
                if g2 < 16:
                    push_qproj(g2 // 4, g2 % 4)
                # o_proj for the previous qt (gated on its 4 norms)
                if pair == 0 and qt >= 1:
                    push_oproj(qt - 1)
                for kt in range(NTT):
                    if qt == 0 and pair == 0:
                        if kt in (0, 4, 8):
                            push_kproj(kt // 4 + 1)
                        if kt in (4, 7, 10, 13):
                            push_vproj((kt - 4) // 3)
                        if kt in (4, 8, 12):
                            force_group(f"kp{kt // 4}")
                    e = scores_step(qt, pair, kt)
                    push_ctx(qt, pair, kt, e)
                    drain(ACT_NS - SCORES_COST)
                push_norm(qt, pair)

        # ================= tail =================
        flush_all()
        # warmth bridge: keep the PE HAM clock up through the final
        # normalization gap so o_proj(3) doesn't run cold
        wps2 = ppp.tile([128, 512], F32, tag="pp")
        for _ in range(8):
            nc.tensor.matmul(wps2[:, 0:128], lhsT=ident[:], rhs=ident[:],
                             start=True, stop=True)
        push_oproj(3)
        force_group("op3")


def _get_nc():
    if "nc" not in _CACHE:
        _CACHE["nc"] = _build_nc()
    return _CACHE["nc"]


def _pack_x(x_b):
    """[S, D] -> [128, 4 slab, 16 kt, 512 t]: xT[k*128+p, nt*512+t]."""
    xT = x_b.T.astype(np.float16)                       # [D, S]
    return np.ascontiguousarray(
        xT.reshape(NKT, 128, 4, 512).transpose(1, 2, 0, 3))


def _pack_wq(wq_s):
    """[D, 512] -> [128, 4 mt, 16 kt, 128]: wq[k*128+p, mt*128+m]."""
    w = wq_s.astype(np.float16)
    return np.ascontiguousarray(
        w.reshape(NKT, 128, 4, 128).transpose(1, 2, 0, 3))


def _pack_wkv(w_s):
    """[D, 128] -> [128, 16 kt, 128]: w[k*128+p, m]."""
    w = w_s.astype(np.float16)
    return np.ascontiguousarray(w.reshape(NKT, 128, KO).transpose(1, 0, 2))


def kernel(x, Wq, bq, Wk, bk, Wv, bv, Wo, bo, _trace=False):
    x = np.asarray(x, np.float32)
    Wq = np.asarray(Wq, np.float32)
    bq = np.asarray(bq, np.float32)
    Wk = np.asarray(Wk, np.float32)
    bk = np.asarray(bk, np.float32)
    Wv = np.asarray(Wv, np.float32)
    bv = np.asarray(bv, np.float32)
    Wo = np.asarray(Wo, np.float32)
    bo = np.asarray(bo, np.float32)

    nc = _get_nc()
    in_maps = []
    for r in range(8):
        b, g = divmod(r, 4)
        qsl = slice(g * 512, (g + 1) * 512)
        ksl = slice(g * 128, (g + 1) * 128)
        in_maps.append({
            "xT": _pack_x(x[b]),
            "wq": _pack_wq(Wq[:, qsl]),
            "wk": _pack_wkv(Wk[:, ksl]),
            "wv": _pack_wkv(Wv[:, ksl]),
            "wo": np.ascontiguousarray(Wo[qsl, :].astype(np.float16)),
            "bq2": np.ascontiguousarray(bq[qsl].reshape(4, 128).T),
            "bk2": np.ascontiguousarray(bk[ksl].reshape(128, 1)),
            "bv2": np.ascontiguousarray(bv[ksl].reshape(128, 1)),
        })

    res = run_bass_kernel_spmd(nc, in_maps, list(range(8)), trace=_trace)
    out = np.zeros((2, S, D), np.float64)
    for r in range(8):
        out[r // 4] += res.results[r]["out"].astype(np.float64)
    out += bo.astype(np.float64)
    result = out.astype(np.float32)
    if _trace:
        return result, res
    return result
